# revision 4
# baseline (speedup 1.0000x reference)
"""Trainium2 Bass kernel for CTM sampling (nn_CTM_30846455120449).

Reference computation (bow is unused by the output):
    theta = softmax(alpha)                    # [K]
    B     = softmax(beta, axis=1)             # [K, K]
    L     = cholesky(sigma)                   # [K, K]
    z     = mu + eps @ L.T                    # [N, K]
    eta   = softmax(z @ B, axis=1)            # [N, K]
    gamma = eta * theta + RHO; gamma /= gamma.sum(1, keepdims=True)

Strategy (data-parallel over 8 cores, 16384 rows each):
  * Fold the [K,K]-sized math on host:  C = L.T @ B,  c0 = mu@B + log(theta).
    Logits l_ij = (eps @ C)_ij + c0_j.
  * Bias-in-matmul: sacrifice eps contraction lanes 510/511 (their true
    contribution to l is ~1e-5) and store a two-digit fp8 decomposition of
    kappa*(c0 + ln(SC) - bbar)/128 in C's rows 510/511 with the eps side
    fixed at 128.0.  One scale kappa=2^s serves both C (near fp8 max) and
    the bias digits; the PSUM then holds kappa*(l + lnSC - bbar) with no
    separate bias matmul (saves 512 TensorE cycles/tile).
  * Per 128-row tile: 2 fp8e4 DoubleRow matmuls (eps chunks stationary,
    C chunks moving).  ScalarE then computes q = u8(exp(psum*2^-s + bbar))
    = u8(SC * exp(l)) directly to uint8, batched over _ACT_BATCH PSUM banks
    per activation to amortize the per-instruction overhead.
  * Output is q [rows, K] uint8 (4x fewer bytes than f32 gamma).  The host
    reconstructs gamma = q/(CDEN*rowsum(q)) + K*RHO/CDEN (uniform theta;
    general-theta formula otherwise) — SC cancels in the ratio.  u8
    quantization contributes ~1e-6 relative error (gate is 2e-2).
  * DMAs move 8-tile groups (512KB): input on the SP hardware-DGE queue,
    output on the ACT hardware-DGE queue.  Each partition owns 8
    consecutive output rows => 4KB contiguous u8 runs per partition.
  * Engine budget per 128-row tile: TensorE 2x518c @2.4GHz = 432ns,
    ScalarE (172/4+512)c @1.2GHz = 462ns, DMA (64KB+64KB)/358GBps = 366ns.
"""

import numpy as np
import ml_dtypes

_N = 131072
_K = 512
_RHO = 0.01
_NCORES = 8
_P = 128
_KC = _K // _P          # 4 contraction chunks of 128
_NSHARD = _N // _NCORES  # 16384 rows per core
_NTILES = _NSHARD // _P  # 128 tiles per core

_prog_cache = {}
_trace = False        # set True externally to profile the run
_last_results = None  # BassKernelResults of the most recent run

_G = 8            # row-tiles per DMA group
_ACT_BATCH = 4    # PSUM banks per ScalarE activation (1, 2 or 4)

_FP8T = ml_dtypes.float8_e4m3


def _build_program(ntiles, act_batch, reps=None):
    import concourse.bass as bass
    import concourse.tile as tile
    from concourse import bacc, mybir

    f32 = mybir.dt.float32
    fp8e4 = mybir.dt.float8e4
    u8 = mybir.dt.uint8
    AF = mybir.ActivationFunctionType
    DR = mybir.MatmulPerfMode.DoubleRow
    nshard = ntiles * _P
    G = _G
    ng = ntiles // G
    assert ntiles % G == 0 and G % act_batch == 0

    inv_scale = float(_act_consts[0])
    bbar = float(_act_consts[1])

    nc = bacc.Bacc("TRN2", target_bir_lowering=False, debug=False)
    epsT_d = nc.declare_dram_parameter("epsT", [ng, _P, G, _KC, _P], fp8e4, isOutput=False)
    C_d = nc.declare_dram_parameter("Cmat", [_P, _KC, _K], fp8e4, isOutput=False)
    q_d = nc.declare_dram_parameter("gamma", [nshard, _K], u8, isOutput=True)
    # partition d owns rows [g*1024 + d*8 .. +8): per-partition-contiguous
    # 4KB u8 runs in the row-major output
    gv = q_d[:].rearrange("(ng d t) k -> ng d t k", d=_P, t=G)

    with tile.TileContext(nc) as tc:
        with (
            tc.tile_pool(name="const", bufs=1) as constp,
            tc.tile_pool(name="eps", bufs=4) as epsp,
            tc.tile_pool(name="psum", bufs=max(2, 8 // act_batch),
                         space=bass.MemorySpace.PSUM) as psump,
            tc.tile_pool(name="gout", bufs=3) as goutp,
        ):
            Ct = constp.tile([_P, _KC, _K], fp8e4)
            nc.gpsimd.dma_start(Ct[:], C_d[:])
            bbt = constp.tile([_P, 1], f32)
            nc.vector.memset(bbt[:], bbar)

            import contextlib
            loop_cm = tc.For_i(0, reps) if reps else contextlib.nullcontext()
            with loop_cm:
                for gi in range(ng):
                    egt = epsp.tile([_P, G, _KC, _P], fp8e4, tag="eps")
                    nc.sync.dma_start(egt[:], epsT_d[gi])
                    gbuf = goutp.tile([_P, G, _K], u8, tag="gbuf")

                    for h in range(G // act_batch):
                        ps = psump.tile([_P, act_batch, _K], f32, tag="ps")
                        for tb in range(act_batch):
                            t = h * act_batch + tb
                            nc.tensor.matmul(
                                ps[:, tb, :], egt[:, t, 0:2, :], Ct[:, 0:2, :],
                                start=True, stop=False, perf_mode=DR,
                            )
                            nc.tensor.matmul(
                                ps[:, tb, :], egt[:, t, 2:4, :], Ct[:, 2:4, :],
                                start=False, stop=True, perf_mode=DR,
                            )
                        nc.scalar.activation(
                            gbuf[:, h * act_batch:(h + 1) * act_batch, :],
                            ps[:], AF.Exp, scale=inv_scale, bias=bbt[:],
                        )

                    nc.scalar.dma_start(gv[gi], gbuf[:])
    nc.compile()
    return nc


# (inv_scale, bbar) for the program build; set by _host_prep before building
_act_consts = [1.0, 0.0]


def _softmax_rows(x):
    m = x.max(axis=-1, keepdims=True)
    e = np.exp(x - m)
    return e / e.sum(axis=-1, keepdims=True)


def _host_prep(alpha, beta, sigma, mu, eps):
    """Fold the small parameters; build fp8 C with embedded bias; shard eps."""
    theta = _softmax_rows(alpha.astype(np.float64))            # [K]
    B = _softmax_rows(beta.astype(np.float64))                 # [K, K]
    L = np.linalg.cholesky(sigma.astype(np.float64))           # [K, K]
    C = L.T @ B                                                # [K, K]
    c0 = mu.astype(np.float64) @ B + np.log(theta)             # [K]

    uniform = bool(np.max(np.abs(theta - 1.0 / _K)) < 1e-12)

    fp8 = lambda x: x.astype(_FP8T).astype(np.float64)

    # u8 output scale: q = SC * exp(l) must stay < 255 including the
    # eps-perturbation of the logits (|eps @ C| <~ 7 * max col norm)
    pad = 7.0 * np.sqrt((C * C).sum(axis=0)).max() + 1e-3
    lmax = c0.max() + pad
    SC = 248.0 / np.exp(lmax)
    b = c0 + np.log(SC)
    bbar = float((b.max() + b.min()) / 2.0)
    db = b - bbar
    dbmax = max(float(np.abs(db).max()), 1e-6)

    # kappa = 2^s: C near fp8 max AND bias digit 128*d0 covers kappa*db
    maxC = float(np.abs(C).max())
    s_C = int(np.floor(np.log2(200.0 / maxC))) if maxC > 0 else 20
    s_b = int(np.floor(np.log2(200.0 * 128.0 / dbmax)))
    s = min(s_C, s_b)
    kappa = 2.0 ** s

    # C chunk layout [P, KC, K]: element [p, c, j] = C[c*P + p, j]
    Cq = fp8(C * kappa)
    d0 = fp8(db * kappa / 128.0)
    d1 = fp8(db * kappa / 128.0 - d0)
    Cq[_K - 2, :] = d0          # contraction lane 510 = (c=3, p=126)
    Cq[_K - 1, :] = d1          # contraction lane 511 = (c=3, p=127)
    Cb = np.ascontiguousarray(
        Cq.reshape(_KC, _P, _K).transpose(1, 0, 2)
    ).astype(_FP8T)

    _act_consts[0] = float(2.0 ** -s)
    _act_consts[1] = bbar

    shards = [
        _prep_eps_shard(eps[core * _NSHARD:(core + 1) * _NSHARD])
        for core in range(_NCORES)
    ]
    return Cb, theta, uniform, s, shards


def _prep_eps_shard(sh):
    """[rows, K] -> [ng, P(k-sub), G(tile), KC, P(doc-lane)].

    Row assignment: lane d of sub-tile t in group g covers row
    g*1024 + d*8 + t, so each partition's group output is 8 consecutive
    rows (one contiguous 4KB u8 DMA run).  Columns 510/511 carry the
    bias-injection constant 128.0 instead of eps."""
    ntiles = sh.shape[0] // _P
    ng = ntiles // _G
    shq = sh.astype(_FP8T)
    shq[:, _K - 2:] = _FP8T(128.0)
    sh5 = shq.reshape(ng, _P, _G, _KC, _P)                # [g, d, t, c, p]
    return np.ascontiguousarray(sh5.transpose(0, 4, 2, 3, 1))


def kernel(bow, alpha, beta, sigma, mu, eps):
    from concourse.bass_utils import run_bass_kernel_spmd

    Cb, theta, uniform, s, shards = _host_prep(alpha, beta, sigma, mu, eps)

    key = (_NTILES, _ACT_BATCH, s, _act_consts[1])
    if key not in _prog_cache:
        _prog_cache[key] = _build_program(_NTILES, _ACT_BATCH)
    nc = _prog_cache[key]

    in_maps = [{"epsT": shards[core], "Cmat": Cb} for core in range(_NCORES)]

    global _last_results
    res = run_bass_kernel_spmd(nc, in_maps, list(range(_NCORES)), trace=_trace)
    _last_results = res
    q = np.concatenate([res.results[i]["gamma"] for i in range(_NCORES)], axis=0)

    # gamma = (q*theta_j + RHO*Q_i) / (sum_j q*theta_j + K*RHO*Q_i); SC cancels
    Q = q.sum(axis=1, dtype=np.int32).astype(np.float32)       # [N]
    qf = q.astype(np.float32)
    if uniform:
        CDEN = np.float32(1.0 + _K * _K * _RHO)
        out = qf * (np.float32(1.0) / (CDEN * Q))[:, None]
        out += np.float32(_K * _RHO / (1.0 + _K * _K * _RHO))
    else:
        th32 = theta.astype(np.float32)
        w = qf * th32[None, :]
        W = w.sum(axis=1)
        out = (w + np.float32(_RHO) * Q[:, None]) / (
            W + np.float32(_K * _RHO) * Q)[:, None]
    return np.ascontiguousarray(out.astype(np.float32))


# revision 23
# speedup vs baseline: 56.0831x; 56.0831x over previous
"""Trainium2 Bass kernel for CTM sampling (nn_CTM_30846455120449).

Reference computation (bow is unused by the output):
    theta = softmax(alpha); B = softmax(beta, 1); L = chol(sigma)
    z = mu + eps @ L.T; eta = softmax(z @ B, 1)
    gamma = eta*theta + RHO; gamma /= gamma.sum(1, keepdims=True)

Strategy (data-parallel over 8 cores, 16384 rows each):
  * Fold [K,K] math on host: C = L.T@B, c0 = mu@B + log(theta).  Logits
    l_ij = (eps@C)_ij + c0_j;  e' = exp(l);  gamma from e' and rowsum(e').
  * Bias-in-matmul: eps contraction lanes 510/511 (true contribution ~1e-5)
    are replaced by the constant 128.0 and C rows 510/511 carry a 2-digit
    fp8 decomposition of the per-column bias, so PSUM = kappa*(l - bshift)
    with no separate bias matmul.  The host knows the exact fp8 digit
    values, so any digit quantization is corrected per column afterwards.
  * Matmuls: 2 fp8e4 DoubleRowSwInterleave matmuls per 128-row tile.  The
    stationary eps tile is stored in the HW SwInterleave layout (pair
    lanes interleaved per out-column, columns reversed); the moving C
    operand is stored pair-interleaved so the streamer reads consecutive
    bytes.  Verified bit-identical to the DoubleRow path.
  * PSUM evacuation is split between both post-processing engines:
      - tiles t in [0, G-nlin): ScalarE  q = u8(exp(ps*2^-s + bbar))
        (true exp; 1 elem/cycle @1.2GHz, (172+512)c per tile)
      - tiles t in [G-nlin, G): VectorE  q = u8(ps * 2^-s2) where the
        moving matrix for these tiles is C'_kj = C_kj*A_j (A = SC*e^{c0}),
        i.e. first-order exp(c0+delta) = A*(1+delta), exact to ~4e-7
        (|delta| <= 6e-4) -- 40x below the u8 quantization step.
    Both engines run concurrently on different PSUM banks; each stays
    under the DMA roofline.
  * Output is q [rows, K] uint8 (4x fewer bytes than f32 gamma).  The host
    rescales per column (two known scale vectors), row-normalizes, and
    assembles f32 gamma.  End-to-end rel err ~1e-6 (gate 2e-2).
  * I/O per core: 8.39MB fp8 in + 8.39MB u8 out = 16.8MB @ ~358GB/s
    => ~47us memory roofline.
"""

import numpy as np
import ml_dtypes

_N = 131072
_K = 512
_RHO = 0.01
_NCORES = 8
_P = 128
_KC = _K // _P          # 4 contraction chunks of 128
_NSHARD = _N // _NCORES  # 16384 rows per core
_NTILES = _NSHARD // _P  # 128 tiles per core

_prog_cache = {}
_trace = False        # set True externally to profile the run
_last_results = None  # BassKernelResults of the most recent run

_G = 8            # row-tiles per DMA group
_NLIN = 0         # tiles per group evacuated by VectorE (linearized path);
#                   0: DVE PSUM-source ops measure ~3x the cost model
#                   (silicon errata), so the split doesn't pay
_PARTS = ("in", "mmsw", "act", "out", "ci", "esw")

_FP8T = ml_dtypes.float8_e4m3


def _build_program(ntiles, nlin=_NLIN, reps=None, parts=_PARTS):
    import concourse.bass as bass
    import concourse.tile as tile
    from concourse import bacc, mybir

    f32 = mybir.dt.float32
    fp8e4 = mybir.dt.float8e4
    u8 = mybir.dt.uint8
    AF = mybir.ActivationFunctionType
    OP = mybir.AluOpType
    nshard = ntiles * _P
    G = _G
    ng = ntiles // G
    assert ntiles % G == 0

    sw = "mmsw" in parts
    ci = "ci" in parts
    esw = "esw" in parts
    pm = (mybir.MatmulPerfMode.DoubleRowSwInterleave if sw
          else mybir.MatmulPerfMode.DoubleRow)
    assert esw == sw, "SwInterleave needs the esw eps layout and vice versa"

    inv_scale = float(_act_consts[0])
    bbar = float(_act_consts[1])
    inv_scale2 = float(_act_consts[2])

    nc = bacc.Bacc("TRN2", target_bir_lowering=False, debug=False)
    if esw:
        epsT_d = nc.declare_dram_parameter("epsT3", [ng, _P, G, 2, _P, 2], fp8e4, isOutput=False)
    else:
        epsT_d = nc.declare_dram_parameter("epsT", [ng, _P, G, _KC, _P], fp8e4, isOutput=False)
    if ci:
        C_d = nc.declare_dram_parameter("Cmat2", [_P, 2, _K, 2], fp8e4, isOutput=False)
        if nlin:
            Cl_d = nc.declare_dram_parameter("Clin2", [_P, 2, _K, 2], fp8e4, isOutput=False)
    else:
        C_d = nc.declare_dram_parameter("Cmat", [_P, _KC, _K], fp8e4, isOutput=False)
        if nlin:
            Cl_d = nc.declare_dram_parameter("Clin", [_P, _KC, _K], fp8e4, isOutput=False)
    q_d = nc.declare_dram_parameter("gamma", [nshard, _K], u8, isOutput=True)
    # partition d owns rows [g*1024 + d*8 .. +8): per-partition-contiguous
    # 4KB u8 runs in the row-major output
    gv = q_d[:].rearrange("(ng d t) k -> ng d t k", d=_P, t=G)

    with tile.TileContext(nc) as tc:
        with (
            tc.tile_pool(name="const", bufs=1) as constp,
            tc.tile_pool(name="eps", bufs=4) as epsp,
            tc.tile_pool(name="psum", bufs=8, space=bass.MemorySpace.PSUM) as psump,
            tc.tile_pool(name="gout", bufs=3) as goutp,
        ):
            cshape = [_P, 2, _K, 2] if ci else [_P, _KC, _K]
            Ct = constp.tile(cshape, fp8e4)
            nc.gpsimd.dma_start(Ct[:], C_d[:])
            if nlin:
                Ctl = constp.tile(cshape, fp8e4)
                nc.gpsimd.dma_start(Ctl[:], Cl_d[:])
            bbt = constp.tile([_P, 1], f32)
            nc.vector.memset(bbt[:], bbar)

            def movings(tile_):
                if ci:
                    return [tile_[:, cp, :, :].rearrange("p j r -> p r j")
                            for cp in (0, 1)]
                return [tile_[:, 0:2, :], tile_[:, 2:4, :]]

            import contextlib
            loop_cm = tc.For_i(0, reps) if reps else contextlib.nullcontext()
            with loop_cm:
                for gi in range(ng):
                    egt = epsp.tile([_P, G, 2, _P, 2] if esw
                                    else [_P, G, _KC, _P], fp8e4, tag="eps")
                    if "in" in parts:
                        nc.sync.dma_start(egt[:], epsT_d[gi])
                    gbuf = goutp.tile([_P, G, _K], u8, tag="gbuf")

                    for t in range(G):
                        lin = t >= G - nlin
                        ps = psump.tile([_P, _K], f32, tag="ps")
                        if esw:
                            lhs = [egt[:, t, cp, :, :] for cp in (0, 1)]
                        else:
                            lhs = [egt[:, t, 0:2, :], egt[:, t, 2:4, :]]
                        rhs = movings(Ctl if lin else Ct)
                        if "mm" in parts or "mmsw" in parts:
                            nc.tensor.matmul(ps[:], lhs[0], rhs[0],
                                             start=True, stop=False, perf_mode=pm)
                            nc.tensor.matmul(ps[:], lhs[1], rhs[1],
                                             start=False, stop=True, perf_mode=pm)
                        if "act" in parts:
                            if lin:
                                nc.vector.tensor_scalar(
                                    gbuf[:, t, :], ps[:], inv_scale2, 0.0,
                                    OP.mult, OP.add)
                            else:
                                nc.scalar.activation(
                                    gbuf[:, t, :], ps[:], AF.Exp,
                                    scale=inv_scale, bias=bbt[:])

                    if "out" in parts:
                        nc.scalar.dma_start(gv[gi], gbuf[:])
    nc.compile()
    return nc


# (inv_scale, bbar, inv_scale2) for the program build; set by _host_prep
_act_consts = [1.0, 0.0, 1.0]


def _softmax_rows(x):
    m = x.max(axis=-1, keepdims=True)
    e = np.exp(x - m)
    return e / e.sum(axis=-1, keepdims=True)


def _fp8r(x):
    return np.asarray(x).astype(_FP8T).astype(np.float64)


def _host_prep(alpha, beta, sigma, mu, eps):
    """Fold the small parameters; build the fp8 C matrices; shard eps.

    Returns (consts, meta, shards):
      consts: dict of device parameter arrays (C variants)
      meta:   reconstruction data (column scales, theta, uniform flag)
    """
    theta = _softmax_rows(alpha.astype(np.float64))            # [K]
    B = _softmax_rows(beta.astype(np.float64))                 # [K, K]
    L = np.linalg.cholesky(sigma.astype(np.float64))           # [K, K]
    C = L.T @ B                                                # [K, K]
    c0 = mu.astype(np.float64) @ B + np.log(theta)             # [K]

    uniform = bool(np.max(np.abs(theta - 1.0 / _K)) < 1e-12)

    # u8 scale: q = SC*exp(l) < 255 including the eps part of the logits
    pad = 7.0 * np.sqrt((C * C).sum(axis=0)).max() + 1e-3
    SC = 248.0 / np.exp(c0.max() + pad)
    b = c0 + np.log(SC)
    bbar = float((b.max() + b.min()) / 2.0)
    db = b - bbar
    dbmax = max(float(np.abs(db).max()), 1e-6)

    # exp path: kappa = 2^s with C near fp8 max and 128*d0 covering kappa*db
    maxC = float(np.abs(C).max())
    s_C = int(np.floor(np.log2(200.0 / maxC))) if maxC > 0 else 20
    s_b = int(np.floor(np.log2(200.0 * 128.0 / dbmax)))
    s = min(s_C, s_b)
    kappa = 2.0 ** s

    Cq = _fp8r(C * kappa)
    d0 = _fp8r(db * kappa / 128.0)
    d1 = _fp8r(db * kappa / 128.0 - d0)
    Cq[_K - 2, :] = d0
    Cq[_K - 1, :] = d1
    b_eff = 128.0 * (d0 + d1) / kappa          # bias the device actually adds
    cexp = np.exp(db - b_eff)                  # per-column correction -> SC*e^l units

    # linear path: PSUM = kappa2*(A_j + A_j*delta), A = SC*e^{c0}
    A = SC * np.exp(c0)                        # [K] in (0, 248]
    s2 = int(np.floor(np.log2(240.0 * 256.0 / (A.max() * 1.0001))))
    kappa2 = 2.0 ** s2
    d0l = _fp8r(A * kappa2 / 256.0)
    d1l = _fp8r(A * kappa2 / 128.0 - d0l)
    A_eff = 128.0 * (d0l + d1l) / kappa2
    Cl = _fp8r(C * A[None, :] * kappa2)
    Cl[_K - 2, :] = d0l
    Cl[_K - 1, :] = d1l
    clin = (SC * np.exp(c0)) / np.maximum(A_eff, 1e-30)

    def layouts(M):
        l1 = np.ascontiguousarray(
            M.reshape(_KC, _P, _K).transpose(1, 0, 2)).astype(_FP8T)
        l2 = np.ascontiguousarray(
            M.reshape(2, 2, _P, _K).transpose(2, 0, 3, 1)).astype(_FP8T)
        return l1, l2

    Cb, Cb2 = layouts(Cq)
    Clb, Clb2 = layouts(Cl)

    _act_consts[0] = float(2.0 ** -s)
    _act_consts[1] = bbar
    _act_consts[2] = float(2.0 ** -s2)

    consts = {"Cmat": Cb, "Cmat2": Cb2, "Clin": Clb, "Clin2": Clb2}
    meta = {"cexp": cexp.astype(np.float32), "clin": clin.astype(np.float32),
            "theta": theta.astype(np.float32), "uniform": uniform,
            "key": (s, s2, bbar)}
    shards = [
        _prep_eps_shard(eps[core * _NSHARD:(core + 1) * _NSHARD])
        for core in range(_NCORES)
    ]
    return consts, meta, shards


def _prep_eps_shard(sh):
    """[rows, K] -> plain DoubleRow layout and SwInterleave layout.

    Row assignment: lane d of sub-tile t in group g covers row
    g*1024 + d*8 + t, so each partition's group output is 8 consecutive
    rows (one contiguous 4KB u8 DMA run).  Columns 510/511 carry the
    bias-injection constant 128.0 instead of eps."""
    ntiles = sh.shape[0] // _P
    ng = ntiles // _G
    shq = sh.astype(_FP8T)
    shq[:, _K - 2:] = _FP8T(128.0)
    sh5 = shq.reshape(ng, _P, _G, _KC, _P)                # [g, d, t, c, p]
    e1 = np.ascontiguousarray(sh5.transpose(0, 4, 2, 3, 1))
    # SwInterleave weights layout: per partition row A127,B127,...,A0,B0
    # [g, p, t, cp, m, r] = eps[row(d=127-m), (2cp+r)*128+p]
    sh6 = shq.reshape(ng, _P, _G, 2, 2, _P)               # [g, d, t, cp, r, p]
    e3 = np.ascontiguousarray(sh6[:, ::-1].transpose(0, 5, 2, 3, 1, 4))
    return e1, e3


def _reconstruct(q, meta, nlin=_NLIN):
    """q [N, K] u8 -> gamma [N, K] f32 on host."""
    n = q.shape[0]
    e = q.astype(np.float32).reshape(-1, _G, _K)
    e[:, :_G - nlin, :] *= meta["cexp"][None, None, :]
    if nlin:
        e[:, _G - nlin:, :] *= meta["clin"][None, None, :]
    e = e.reshape(n, _K)                                   # common e'-units
    th = meta["theta"]
    if meta["uniform"]:
        T = e.sum(axis=1)
        CDEN = np.float32(1.0 + _K * _K * _RHO)
        out = e * (np.float32(1.0) / (CDEN * T))[:, None]
        out += np.float32(_K * _RHO / (1.0 + _K * _K * _RHO))
    else:
        w = e * th[None, :]
        W = w.sum(axis=1)
        Q = e.sum(axis=1)
        out = (w + np.float32(_RHO) * Q[:, None]) / (
            W + np.float32(_K * _RHO) * Q)[:, None]
    return np.ascontiguousarray(out.astype(np.float32))


def kernel(bow, alpha, beta, sigma, mu, eps):
    from concourse.bass_utils import run_bass_kernel_spmd

    consts, meta, shards = _host_prep(alpha, beta, sigma, mu, eps)

    key = (_NTILES, _NLIN, _PARTS, meta["key"])
    if key not in _prog_cache:
        _prog_cache[key] = _build_program(_NTILES, _NLIN)
    nc = _prog_cache[key]

    eidx = 1 if "esw" in _PARTS else 0
    in_maps = []
    for core in range(_NCORES):
        m = {"epsT3" if eidx else "epsT": shards[core][eidx]}
        m["Cmat2" if "ci" in _PARTS else "Cmat"] = \
            consts["Cmat2" if "ci" in _PARTS else "Cmat"]
        if _NLIN:
            m["Clin2" if "ci" in _PARTS else "Clin"] = \
                consts["Clin2" if "ci" in _PARTS else "Clin"]
        in_maps.append(m)

    global _last_results
    res = run_bass_kernel_spmd(nc, in_maps, list(range(_NCORES)), trace=_trace)
    _last_results = res
    q = np.concatenate([res.results[i]["gamma"] for i in range(_NCORES)], axis=0)
    return _reconstruct(q, meta, _NLIN)
